# revision 12
# baseline (speedup 1.0000x reference)
"""AttentionBlock (groupnorm -> qkv -> softmax attention -> proj -> residual)
on 8 TRN2 NeuronCores, data-parallel over batch (B=32 -> 4 per core).

Self-contained: hardcodes shapes; builds one Bass/Tile graph and runs it
SPMD on cores 0..7 via run_bass_kernel_spmd. Host-side prep (part of the
sharding step): weights pre-cast to fp8e4m3, x additionally passed
pre-transposed (channel-major) in bf16 — pure layout/dtype transforms; all
math runs on device. All five GEMMs run in fp8 with DoubleRow perf mode
(two 128-row k-tiles per matmul). Softmax exp is computed with a constant
shift (exp(s*scale - SHIFT)) so pt stays inside fp8e4m3 range; the shift
cancels exactly in the softmax normalization.
"""

import numpy as np
import ml_dtypes
from contextlib import ExitStack

import concourse.bass as bass
import concourse.tile as tile
from concourse import bacc, mybir
from concourse.bass_utils import run_bass_kernel_spmd

F32 = mybir.dt.float32
BF16 = mybir.dt.bfloat16
F8 = mybir.dt.float8e4
DR = mybir.MatmulPerfMode.DoubleRow

B, H, W, C = 32, 32, 32, 512
N = H * W            # 1024 tokens
G = 8                # groups
NCORES = 8
BPC = B // NCORES    # batches per core
EPS = 1e-3
SCALE = 1.0 / float(np.sqrt(C))
SHIFT = 3.0          # softmax exp shift (cancels in normalization)
P = 128
CT = C // P          # 4 channel tiles
TT = N // P          # 8 token tiles
MQK = 2 * C // P     # 8 d-tiles for q+k


def _build(ctx: ExitStack, tc: "tile.TileContext", io: dict):
    nc = tc.nc
    x_ext = io["x"]            # [BPC, N, C] f32 (residual source)
    xT_ext = io["xT16"]        # [BPC, C, N] bf16 (pre-transposed)
    bqkv_ext = io["b_qkv"]     # [3C] f32
    wqkv_ext = io["wqkv8"]     # [C, 3C] fp8
    wp_ext = io["wp8"]         # [C, C] fp8
    gamma_ext = io["gamma"]    # [C] f32
    beta_ext = io["beta"]      # [C] f32
    bqkv16_ext = io["bqkv16"]  # [3C] bf16
    bp16_ext = io["bp16"]      # [C] bf16
    out_ext = io["out"]        # [BPC, N, C] f32

    # ---------------- pools ----------------
    const_pool = ctx.enter_context(tc.tile_pool(name="consts", bufs=1))
    xf_pool = ctx.enter_context(tc.tile_pool(name="xf", bufs=2))
    xT_pool = ctx.enter_context(tc.tile_pool(name="xT", bufs=2))
    hT_pool = ctx.enter_context(tc.tile_pool(name="hT", bufs=2))
    qk_pool = ctx.enter_context(tc.tile_pool(name="qk", bufs=2))
    v_pool = ctx.enter_context(tc.tile_pool(name="vv", bufs=2))
    pt_pool = ctx.enter_context(tc.tile_pool(name="pt", bufs=2))
    hTn_pool = ctx.enter_context(tc.tile_pool(name="hTn", bufs=2))
    out_pool = ctx.enter_context(tc.tile_pool(name="outb", bufs=2))
    small = ctx.enter_context(tc.tile_pool(name="small", bufs=4))
    tiny = ctx.enter_context(tc.tile_pool(name="tiny", bufs=1))
    rb_pool = ctx.enter_context(tc.tile_pool(name="rb", bufs=2))

    psA = ctx.enter_context(tc.tile_pool(name="psA", bufs=6, space="PSUM"))
    psB = ctx.enter_context(tc.tile_pool(name="psB", bufs=2, space="PSUM"))

    def x_chain(b):
        """load x[b] twice: token-major f32 (residual) + channel-major bf16."""
        xf = xf_pool.tile([P, TT, C], F32, name=f"xf{b}", tag="xf")
        nc.gpsimd.dma_start(out=xf,
                            in_=x_ext[b].rearrange("(t p) c -> p t c", p=P))
        xT = xT_pool.tile([P, CT, N], BF16, name=f"xT{b}", tag="xT")
        nc.sync.dma_start(out=xT,
                          in_=xT_ext[b].rearrange("(ct p) n -> p ct n", p=P))
        return xf, xT

    # batch 0's inputs issue ahead of the weight stream
    xchains = {0: x_chain(0)}

    # ---------------- constants / weights (fp8, direct) ----------------
    wqkv = const_pool.tile([P, CT, 3 * C], F8)
    nc.sync.dma_start(out=wqkv, in_=wqkv_ext.rearrange("(kt p) d -> p kt d", p=P))
    wp = const_pool.tile([P, CT, C], F8)
    nc.sync.dma_start(out=wp, in_=wp_ext.rearrange("(kt p) d -> p kt d", p=P))

    bv16 = const_pool.tile([1, C], BF16)
    nc.gpsimd.dma_start(
        out=bv16,
        in_=bass.AP(tensor=bqkv16_ext.tensor, offset=bqkv16_ext.offset + 2 * C,
                    ap=[[0, 1], [1, C]]),
    )
    bp16 = const_pool.tile([1, C], BF16)
    nc.gpsimd.dma_start(
        out=bp16,
        in_=bass.AP(tensor=bp16_ext.tensor, offset=bp16_ext.offset,
                    ap=[[0, 1], [1, C]]),
    )

    # gamma/beta as [128, CT] f32 (per-channel, channel-partition layout)
    gamma_sb = const_pool.tile([P, CT], F32)
    nc.gpsimd.dma_start(
        out=gamma_sb,
        in_=bass.AP(tensor=gamma_ext.tensor, offset=gamma_ext.offset,
                    ap=[[1, P], [P, CT]]),
    )
    beta_sb = const_pool.tile([P, CT], F32)
    nc.gpsimd.dma_start(
        out=beta_sb,
        in_=bass.AP(tensor=beta_ext.tensor, offset=beta_ext.offset,
                    ap=[[1, P], [P, CT]]),
    )

    # ones helpers
    ones_1x128 = const_pool.tile([1, P], BF16)
    nc.vector.memset(ones_1x128, 1.0)
    ones8_128x1 = const_pool.tile([P, 1], F8)
    nc.vector.memset(ones8_128x1, 1.0)

    # group mask [128, 2]: partition p -> group p//64, value 1/64 (mean-of-64)
    gmask_np = np.zeros((P, 2), dtype=np.float32)
    gmask_np[0:64, 0] = 1.0 / 64.0
    gmask_np[64:128, 1] = 1.0 / 64.0
    gmask = const_pool.tile([P, 2], F32)
    nc.gpsimd.dma_start(out=gmask, in_=nc.inline_tensor(gmask_np, "gmask_c").ap())
    # broadcast-back mask [2, 128]: maskT[r, p] = (p//64 == r)
    bmaskT_np = np.zeros((2, P), dtype=np.float32)
    bmaskT_np[0, 0:64] = 1.0
    bmaskT_np[1, 64:128] = 1.0
    bmaskT = const_pool.tile([2, P], F32)
    nc.gpsimd.dma_start(out=bmaskT, in_=nc.inline_tensor(bmaskT_np, "bmaskT_c").ap())
    eps_sb = const_pool.tile([2, 1], F32)
    nc.vector.memset(eps_sb, EPS)
    shift_sb = const_pool.tile([P, 1], F32)
    nc.vector.memset(shift_sb, -SHIFT)

    # b_qkv[0:1024] as per-partition columns [128, MQK] f32 (qkT copy-out bias)
    bqk_cols = const_pool.tile([P, MQK], F32)
    nc.gpsimd.dma_start(
        out=bqk_cols,
        in_=bass.AP(tensor=bqkv_ext.tensor, offset=bqkv_ext.offset,
                    ap=[[1, P], [P, MQK]]),
    )
    # broadcast b_v and b_proj across 128 partitions (one-time ones-matmuls)
    bv_bcast = const_pool.tile([P, C], BF16)
    bp_bcast = const_pool.tile([P, C], F32)
    ps_bc = psA.tile([P, 512], F32, tag="ps")
    nc.tensor.matmul(ps_bc, lhsT=ones_1x128, rhs=bv16, start=True, stop=True)
    nc.scalar.copy(bv_bcast, ps_bc)
    ps_bc2 = psA.tile([P, 512], F32, tag="ps")
    nc.tensor.matmul(ps_bc2, lhsT=ones_1x128, rhs=bp16, start=True, stop=True)
    nc.scalar.copy(bp_bcast, ps_bc2)

    for b in range(BPC):
        # ---------------- load ----------------
        xf, xT = xchains[b] if b in xchains else x_chain(b)
        # fold b_proj into the residual source: xf += b_proj (broadcast)
        nc.vector.tensor_add(
            xf, xf,
            bass.AP(tensor=bp_bcast.tensor, offset=bp_bcast.offset,
                    ap=[bp_bcast.ap[0], [0, TT], [1, C]]),
        )

        # ---------------- groupnorm stats (batched across c-tiles) --------
        mv = small.tile([P, CT, 2], F32, tag="mv")  # per-channel [mean, var]
        for ct in range(CT):
            st = small.tile([P, 2, 6], F32, tag="st")
            nc.vector.bn_stats(st[:, 0, :], xT[:, ct, 0:512])
            nc.vector.bn_stats(st[:, 1, :], xT[:, ct, 512:1024])
            nc.vector.bn_aggr(mv[:, ct, :], st)
        q2 = small.tile([P, CT, 2], F32, tag="q2")  # [mean, E[x^2]]
        nc.vector.tensor_mul(q2[:, :, 1], mv[:, :, 0], mv[:, :, 0])
        nc.vector.tensor_add(q2[:, :, 1], q2[:, :, 1], mv[:, :, 1])
        nc.vector.tensor_copy(q2[:, :, 0], mv[:, :, 0])
        ps_st = psB.tile([2, 8], F32, tag="ps_small")  # [g, (ct, stat)]
        nc.tensor.matmul(ps_st, lhsT=gmask, rhs=q2, start=True, stop=True)

        st_sb = small.tile([2, CT, 2], F32, tag="st_sb")
        nc.vector.tensor_copy(st_sb, ps_st)
        gmean = st_sb[:, :, 0]    # [2, 4] group means
        gm2 = st_sb[:, :, 1]      # [2, 4] group E[x^2]
        rsm = small.tile([2, CT, 2], F32, tag="rsm")  # [:,ct,0]=rstd [:,ct,1]=mean
        var24 = rsm[:, :, 0]
        nc.vector.tensor_mul(var24, gmean, gmean)
        nc.vector.tensor_sub(var24, gm2, var24)
        # rstd = exp(-0.5*ln(var+eps)) — Ln/Exp share one ACT table set with
        # softmax's Exp, so no per-batch table reloads
        nc.scalar.activation(var24, var24, mybir.ActivationFunctionType.Ln,
                             bias=eps_sb, scale=1.0)
        nc.scalar.activation(var24, var24, mybir.ActivationFunctionType.Exp,
                             scale=-0.5)
        nc.vector.tensor_copy(rsm[:, :, 1], gmean)

        ps_pc = psB.tile([P, CT, 2], F32, tag="ps_small")  # [rstd_c, mean_c]
        nc.tensor.matmul(ps_pc, lhsT=bmaskT, rhs=rsm, start=True, stop=True)
        A_sb = small.tile([P, CT], F32, tag="A")
        B_sb = small.tile([P, CT], F32, tag="B")
        nc.vector.tensor_mul(A_sb, ps_pc[:, :, 0], gamma_sb)
        nc.vector.tensor_mul(B_sb, ps_pc[:, :, 1], A_sb)
        nc.vector.tensor_sub(B_sb, beta_sb, B_sb)

        # ---------------- normalize: hT = xT*A + B (fp8) ----------------
        hT = hT_pool.tile([P, CT, N], F8, name=f"hT{b}", tag="hT")
        for ct in range(CT):
            nc.vector.tensor_scalar(
                out=hT[:, ct, :], in0=xT[:, ct, :],
                scalar1=A_sb[:, ct:ct + 1], scalar2=B_sb[:, ct:ct + 1],
                op0=mybir.AluOpType.mult, op1=mybir.AluOpType.add,
            )

        # ---------------- qkT = (W_qk)^T @ hT  [d-major, fp8 DR] ----------
        qk = qk_pool.tile([P, MQK, N], F8, name=f"qk{b}", tag="qk")
        for m in range(MQK):
            ps0 = psA.tile([P, 512], F32, tag="ps")
            ps1 = psA.tile([P, 512], F32, tag="ps")
            for kc in (0, 2):
                lw = wqkv[:, kc:kc + 2, m * P:(m + 1) * P]
                nc.tensor.matmul(ps0, lhsT=lw, rhs=hT[:, kc:kc + 2, 0:512],
                                 start=(kc == 0), stop=(kc == 2), perf_mode=DR)
                nc.tensor.matmul(ps1, lhsT=lw, rhs=hT[:, kc:kc + 2, 512:1024],
                                 start=(kc == 0), stop=(kc == 2), perf_mode=DR)
            nc.scalar.activation(qk[:, m, 0:512], ps0,
                                 mybir.ActivationFunctionType.Identity,
                                 bias=bqk_cols[:, m:m + 1])
            nc.scalar.activation(qk[:, m, 512:1024], ps1,
                                 mybir.ActivationFunctionType.Identity,
                                 bias=bqk_cols[:, m:m + 1])

        # ---------------- v = hT^T @ W_v  [token-major, fp8 DR] -----------
        vv = v_pool.tile([P, TT, C], F8, name=f"vv{b}", tag="vv")
        for m in range(TT):
            ps = psA.tile([P, 512], F32, tag="ps")
            for kc in (0, 2):
                nc.tensor.matmul(ps, lhsT=hT[:, kc:kc + 2, m * P:(m + 1) * P],
                                 rhs=wqkv[:, kc:kc + 2, 1024:1536],
                                 start=(kc == 0), stop=(kc == 2), perf_mode=DR)
            nc.vector.tensor_add(vv[:, m, :], ps, bv_bcast)

        # ---- attention pipelined per query-half: scores -> exp -> r ->
        # ---- attnv -> hTn -> proj -> out.  Halves are independent past
        # ---- qk/vv, so half 1's scores/exp overlap half 0's attnv/proj.
        pt = pt_pool.tile([P, TT, N], F8, name=f"pt{b}", tag="pt")
        hTn = hTn_pool.tile([P, CT, N], F8, name=f"hTn{b}", tag="hTn")
        rb = rb_pool.tile([P, N], F32, name=f"rb{b}", tag="rb")
        outb = out_pool.tile([P, TT, C], F32, name=f"outb{b}", tag="outb")
        for h in range(2):
            sl = slice(h * 512, (h + 1) * 512)
            # scores + exp
            for mk in range(TT):
                ps = psA.tile([P, 512], F32, tag="ps")
                for cc in (0, 2):
                    lw = qk[:, 4 + cc:4 + cc + 2, mk * P:(mk + 1) * P]  # kT
                    nc.tensor.matmul(ps, lhsT=lw, rhs=qk[:, cc:cc + 2, sl],
                                     start=(cc == 0), stop=(cc == 2),
                                     perf_mode=DR)
                nc.scalar.activation(pt[:, mk, sl], ps,
                                     mybir.ActivationFunctionType.Exp,
                                     bias=shift_sb, scale=SCALE)
            # softmax denominator r[q] = sum_keys pt
            ps_r = psA.tile([1, 512], F32, tag="ps")
            for mk in range(TT):
                nc.tensor.matmul(ps_r, lhsT=ones8_128x1, rhs=pt[:, mk, sl],
                                 start=(mk == 0), stop=(mk == TT - 1))
            r16 = tiny.tile([1, 512], BF16, tag="r16")
            nc.vector.tensor_copy(r16, ps_r)
            ps_b = psA.tile([P, 512], F32, tag="ps")
            nc.tensor.matmul(ps_b, lhsT=ones_1x128, rhs=r16,
                             start=True, stop=True)
            # 128-partition-parallel fast reciprocal (psum -> sbuf f32)
            nc.vector.reciprocal_approx_fast(out=rb[:, sl], in_=ps_b)

            # hTn = (v^T @ pt) * rb  [channel-major, fp8]
            for mc in range(CT):
                ps = psA.tile([P, 512], F32, tag="ps")
                for mk in (0, 2, 4, 6):
                    lw = vv[:, mk:mk + 2, mc * P:(mc + 1) * P]
                    nc.tensor.matmul(ps, lhsT=lw, rhs=pt[:, mk:mk + 2, sl],
                                     start=(mk == 0), stop=(mk == 6),
                                     perf_mode=DR)
                nc.vector.tensor_mul(hTn[:, mc, sl], ps, rb[:, sl])

            # proj + residual -> out [token-major, fp8 DR]
            for m in range(h * 4, h * 4 + 4):
                ps = psA.tile([P, 512], F32, tag="ps")
                for mc in (0, 2):
                    nc.tensor.matmul(ps,
                                     lhsT=hTn[:, mc:mc + 2, m * P:(m + 1) * P],
                                     rhs=wp[:, mc:mc + 2, :], start=(mc == 0),
                                     stop=(mc == 2), perf_mode=DR)
                nc.vector.tensor_add(outb[:, m, :], ps, xf[:, m, :])
                nc.gpsimd.dma_start(
                    out=out_ext[b].rearrange("(t p) c -> p t c", p=P)[:, m, :],
                    in_=outb[:, m, :])


_CACHED_NC = None


def _build_nc():
    global _CACHED_NC
    if _CACHED_NC is not None:
        return _CACHED_NC
    nc = bacc.Bacc("TRN2", target_bir_lowering=False, debug=False,
                   num_devices=NCORES)
    io = {
        "x": nc.dram_tensor("x", [BPC, N, C], F32, kind="ExternalInput").ap(),
        "xT16": nc.dram_tensor("xT16", [BPC, C, N], BF16,
                               kind="ExternalInput").ap(),
        "gamma": nc.dram_tensor("gamma", [C], F32, kind="ExternalInput").ap(),
        "beta": nc.dram_tensor("beta", [C], F32, kind="ExternalInput").ap(),
        "wqkv8": nc.dram_tensor("wqkv8", [C, 3 * C], F8,
                                kind="ExternalInput").ap(),
        "b_qkv": nc.dram_tensor("b_qkv", [3 * C], F32, kind="ExternalInput").ap(),
        "bqkv16": nc.dram_tensor("bqkv16", [3 * C], BF16,
                                 kind="ExternalInput").ap(),
        "wp8": nc.dram_tensor("wp8", [C, C], F8, kind="ExternalInput").ap(),
        "bp16": nc.dram_tensor("bp16", [C], BF16, kind="ExternalInput").ap(),
        "out": nc.dram_tensor("out", [BPC, N, C], F32, kind="ExternalOutput").ap(),
    }
    with tile.TileContext(nc) as tc:
        with ExitStack() as ctx:
            _build(ctx, tc, io)
    nc.compile()
    _CACHED_NC = nc
    return nc


def _run(inputs: dict, trace: bool = False):
    nc = _build_nc()
    x = np.ascontiguousarray(inputs["x"], dtype=np.float32).reshape(B, N, C)
    xT16_full = np.ascontiguousarray(
        x.transpose(0, 2, 1)).astype(ml_dtypes.bfloat16)  # [B, C, N]
    shared = {
        "gamma": np.ascontiguousarray(inputs["gamma"], dtype=np.float32),
        "beta": np.ascontiguousarray(inputs["beta"], dtype=np.float32),
        "b_qkv": np.ascontiguousarray(inputs["b_qkv"], dtype=np.float32),
        "wqkv8": np.ascontiguousarray(inputs["w_qkv"], dtype=np.float32)
            .astype(ml_dtypes.float8_e4m3),
        "bqkv16": np.ascontiguousarray(inputs["b_qkv"], dtype=np.float32)
            .astype(ml_dtypes.bfloat16),
        "wp8": np.ascontiguousarray(inputs["w_proj"], dtype=np.float32)
            .astype(ml_dtypes.float8_e4m3),
        "bp16": np.ascontiguousarray(inputs["b_proj"], dtype=np.float32)
            .astype(ml_dtypes.bfloat16),
    }
    in_maps = []
    for i in range(NCORES):
        m = {"x": x[i * BPC:(i + 1) * BPC],
             "xT16": xT16_full[i * BPC:(i + 1) * BPC]}
        m.update(shared)
        in_maps.append(m)
    res = run_bass_kernel_spmd(nc, in_maps, list(range(NCORES)), trace=trace)
    outs = [res.results[i]["out"].reshape(BPC, H, W, C) for i in range(NCORES)]
    full = np.concatenate(outs, axis=0)
    return full, res


def kernel(**inputs) -> np.ndarray:
    full, _ = _run(inputs, trace=False)
    return full
